# revision 1
# baseline (speedup 1.0000x reference)
"""Multi-head attention kernel for Trainium2, head-parallel across 8 NeuronCores.

Math per head h (reference):
    scores  = X @ W[h] @ X.T / sqrt(D)          [N, N]
    weights = softmax(scores, axis=-1) + 1e-8
    out    += weights @ (X @ V[h])              [N, D], summed over heads

Sharding: H=40 heads split 5-per-core across 8 cores; X replicated.  Each core
computes the partial sum of its 5 heads' outputs; the host sums the 8 partials.

Per-core kernel layout (all matmuls contract over the partition axis):
    XT   [d, n]   = X^T               (PE transposes, done once)
    XV   [m, e]   = X @ V[h]          (natural layout, lhsT=XT tile)
    XWT  [e, n]   = W[h]^T "@" XT     (lhsT=W[h], rhs=XT)
    scT  [m, n]   = XT_tile^T @ XWT   (scores transposed: m on partitions)
    E    [m, n]   = exp(scT / sqrt(D))     (ACT, PSUM->SBUF)
    rs   [1, n]   = ones^T @ E        (softmax denominator via PE)
    rr   [1, n]   = 1 / rs            (DVE reciprocal)
    bc   [p, n]   = broadcast of rr   (K=1 matmul with ones row)
    avT  [e, n]   = XV_tile^T @ E     (unnormalised attention output, transposed)
    OUT  [e, n]  += avT * bc          (DVE), then PE-transpose to [n, e] at the end

The transposed-scores layout makes the AV contraction (over m) natural and the
softmax denominator is recovered with cheap M=1 / K=1 matmuls.

Matmul operands are stored as float16: full PE rate (1 cycle/row, like bf16)
with a 10-bit mantissa, and every operand here is comfortably inside fp16
range (X ~ N(0,1), exp values in [0.3, 3], XV ~ 0.1).  PSUM accumulation is
fp32 throughout.
"""

import sys

import numpy as np

try:
    import concourse  # noqa: F401  (provided by the container's sitecustomize)
except ImportError:  # pragma: no cover
    for p in ("/opt/trn_rl_repo", "/root/.axon_site/_ro/trn_rl_repo"):
        if p not in sys.path:
            sys.path.insert(0, p)

N, D, H, NCORES = 2048, 128, 40, 8
HC = H // NCORES          # heads per core
NT = N // 128             # 128-row tiles of n/m
CH = N // 512             # 512-column chunks of n
SCALE = 1.0 / float(np.sqrt(np.float32(D)))

# mm: matmul operand dtype, "f16" (default) or "bf16".
# scpsum: scores PSUM dtype — "f16" packs [128,2048] scores into 2 banks so
#         exp runs in 4 big ACT instructions per chunk; "f32" uses [128,1024].
# rowsum: "pe" = 16 ones-matmuls per chunk on the tensor engine;
#         "dve_reduce" = one strided DVE tensor_reduce + a single ones-matmul;
#         "dve_adds" = chain of DVE adds + a single ones-matmul.
CFG = {"mm": "f16", "scpsum": "f32", "rowsum": "dve_adds",
       "scp_bufs": 2, "exp_bufs": 3, "av_bufs": 2, "sched": "chunked",
       "arch": "chunked"}

_CACHE = {}


def _emit_mt_major(ctx, tc, nc, X, W, V, out, cfg):
    """m_tile-major schedule: for each m-tile, all 4 n-chunks' scores share
    one PE weight load (XT tile), the two exp instructions cover 2 chunks
    each, and the 4 AV accumulators (one PSUM bank per chunk) share the XV
    weight load.  Row-sums accumulate on the DVE; the per-chunk softmax
    normalisation tail runs through the scores PSUM pool."""
    from concourse import mybir
    from concourse.masks import make_identity

    f32 = mybir.dt.float32
    mdt = {"f16": mybir.dt.float16, "bf16": mybir.dt.bfloat16}[cfg["mm"]]
    Exp = mybir.ActivationFunctionType.Exp

    consts = ctx.enter_context(tc.tile_pool(name="consts", bufs=1))
    big = ctx.enter_context(tc.tile_pool(name="big", bufs=1))
    xwtp = ctx.enter_context(tc.tile_pool(name="xwtp", bufs=2))
    expp = ctx.enter_context(tc.tile_pool(name="expp", bufs=1))
    eaccp = ctx.enter_context(tc.tile_pool(name="eaccp", bufs=8))
    smallp = ctx.enter_context(tc.tile_pool(name="smallp", bufs=2))
    scp = ctx.enter_context(tc.tile_pool(name="scp", bufs=2, space="PSUM"))
    avp = ctx.enter_context(tc.tile_pool(name="avp", bufs=4, space="PSUM"))

    idt = consts.tile([128, 128], f32, tag="idt")
    make_identity(nc, idt[:])
    ones = consts.tile([128, 128], mdt, tag="ones")
    nc.gpsimd.memset(ones[:], 1.0)

    X_stage = big.tile([128, N], f32, tag="xstage")
    for nt in range(NT):
        nc.sync.dma_start(out=X_stage[:, nt * 128:(nt + 1) * 128],
                          in_=X[nt * 128:(nt + 1) * 128, :])
    XT = big.tile([128, N], mdt, tag="xt")
    for nt in range(NT):
        pt = scp.tile([128, 128], f32, tag="sc", name="pt")
        nc.tensor.transpose(pt[:], X_stage[:, nt * 128:(nt + 1) * 128], idt[:])
        nc.vector.tensor_copy(XT[:, nt * 128:(nt + 1) * 128], pt[:])

    Wf = big.tile([128, HC * 128], f32, tag="wf")
    Vf = big.tile([128, HC * 128], f32, tag="vf")
    for h in range(HC):
        nc.sync.dma_start(out=Wf[:, h * 128:(h + 1) * 128], in_=W[h])
        nc.sync.dma_start(out=Vf[:, h * 128:(h + 1) * 128], in_=V[h])
    Wc = big.tile([128, HC * 128], mdt, tag="wc")
    Vc = big.tile([128, HC * 128], mdt, tag="vc")
    nc.vector.tensor_copy(Wc[:], Wf[:])
    nc.vector.tensor_copy(Vc[:], Vf[:])

    XV = big.tile([128, NT * HC * 128], mdt, tag="xv")
    for mt in range(NT):
        sct = scp.tile([128, 1024], f32, tag="sc", name="sct")
        nc.tensor.matmul(sct[:, 0:512], XT[:, mt * 128:(mt + 1) * 128],
                         Vc[:, 0:512], start=True, stop=True)
        nc.tensor.matmul(sct[:, 512:512 + (HC - 4) * 128],
                         XT[:, mt * 128:(mt + 1) * 128],
                         Vc[:, 512:HC * 128], start=True, stop=True)
        nc.vector.tensor_copy(XV[:, mt * HC * 128:(mt + 1) * HC * 128],
                              sct[:, 0:HC * 128])

    OUT_acc = big.tile([128, N], f32, tag="oacc")

    for h in range(HC):
        XWT = xwtp.tile([128, N], mdt, tag="xwt")
        for g in range(2):
            sct = scp.tile([128, 1024], f32, tag="sc", name="sct")
            for j in range(2):
                c = 2 * g + j
                nc.tensor.matmul(sct[:, j * 512:(j + 1) * 512],
                                 Wc[:, h * 128:(h + 1) * 128],
                                 XT[:, c * 512:(c + 1) * 512],
                                 start=True, stop=True)
            nc.vector.tensor_copy(XWT[:, g * 1024:(g + 1) * 1024], sct[:, 0:1024])

        EXP = expp.tile([128, CH * NT * 512], mdt, tag="exp")
        EXPv = EXP.rearrange("p (c r) -> p c r", c=CH)
        AVs = [avp.tile([128, 512], f32, tag="av", name=f"av{c}")
               for c in range(CH)]
        EACCs = [eaccp.tile([128, 512], f32, tag="eacc", name=f"eacc{c}")
                 for c in range(CH)]
        for mt in range(NT):
            mcol = slice(mt * 512, (mt + 1) * 512)
            sa = scp.tile([128, 1024], f32, tag="sc", name="sa")
            sb = scp.tile([128, 1024], f32, tag="sc", name="sb")
            for j, sct in ((0, sa), (1, sa), (2, sb), (3, sb)):
                nc.tensor.matmul(sct[:, (j % 2) * 512:(j % 2 + 1) * 512],
                                 XT[:, mt * 128:(mt + 1) * 128],
                                 XWT[:, j * 512:(j + 1) * 512],
                                 start=True, stop=True)
            nc.scalar.activation(
                EXPv[:, 0:2, mcol],
                sa[:, 0:1024].rearrange("p (a b) -> p a b", a=2),
                Exp, scale=SCALE)
            nc.scalar.activation(
                EXPv[:, 2:4, mcol],
                sb[:, 0:1024].rearrange("p (a b) -> p a b", a=2),
                Exp, scale=SCALE)
            for c in range(CH):
                nc.tensor.matmul(AVs[c][:],
                                 XV[:, mt * HC * 128 + h * 128:
                                        mt * HC * 128 + (h + 1) * 128],
                                 EXPv[:, c, mcol],
                                 start=(mt == 0), stop=(mt == NT - 1))
            for c in range(CH):
                if mt == 0:
                    nc.vector.tensor_copy(EACCs[c][:], EXPv[:, c, mcol])
                else:
                    nc.vector.tensor_add(EACCs[c][:], EACCs[c][:],
                                         EXPv[:, c, mcol])
        for c in range(CH):
            ncol = slice(c * 512, (c + 1) * 512)
            EACCh = smallp.tile([128, 512], mdt, tag="eacch")
            nc.vector.tensor_copy(EACCh[:], EACCs[c][:])
            RS = scp.tile([1, 512], f32, tag="sc", name="rs")
            nc.tensor.matmul(RS[:], ones[:, 0:1], EACCh[:],
                             start=True, stop=True)
            RSr = smallp.tile([1, 512], f32, tag="rsr")
            nc.vector.reciprocal(RSr[:], RS[:])
            RSh = smallp.tile([1, 512], mdt, tag="rsh")
            nc.vector.tensor_copy(RSh[:], RSr[:])
            BC = scp.tile([128, 512], f32, tag="sc", name="bc")
            nc.tensor.matmul(BC[:], ones[0:1, :], RSh[:], start=True, stop=True)
            BC_sb = smallp.tile([128, 512], f32, tag="bcsb")
            nc.vector.tensor_copy(BC_sb[:], BC[:])
            if h == 0:
                nc.vector.tensor_mul(OUT_acc[:, ncol], AVs[c][:], BC_sb[:])
            else:
                tmp = smallp.tile([128, 512], f32, tag="tmp")
                nc.vector.tensor_mul(tmp[:], AVs[c][:], BC_sb[:])
                nc.vector.tensor_add(OUT_acc[:, ncol], OUT_acc[:, ncol],
                                     tmp[:])

    for nt in range(NT):
        pt = scp.tile([128, 128], f32, tag="sc", name="pt2")
        nc.tensor.transpose(pt[:], OUT_acc[:, nt * 128:(nt + 1) * 128], idt[:])
        OUTN = smallp.tile([128, 128], f32, tag="outn")
        nc.vector.tensor_copy(OUTN[:], pt[:])
        nc.sync.dma_start(out=out[nt * 128:(nt + 1) * 128, :], in_=OUTN[:])


def _emit(ctx, tc, nc, X, W, V, out, cfg):
    if cfg.get("arch") == "mt_major":
        return _emit_mt_major(ctx, tc, nc, X, W, V, out, cfg)
    from concourse import mybir
    from concourse.masks import make_identity

    f32 = mybir.dt.float32
    mdt = {"f16": mybir.dt.float16, "bf16": mybir.dt.bfloat16}[cfg["mm"]]
    Exp = mybir.ActivationFunctionType.Exp

    # ---- pools ----
    consts = ctx.enter_context(tc.tile_pool(name="consts", bufs=1))
    big = ctx.enter_context(tc.tile_pool(name="big", bufs=1))
    xwtp = ctx.enter_context(tc.tile_pool(name="xwtp", bufs=2))
    expp = ctx.enter_context(tc.tile_pool(name="expp", bufs=cfg["exp_bufs"]))
    smallp = ctx.enter_context(tc.tile_pool(name="smallp", bufs=2))
    gp_rowsum = cfg["rowsum"] == "gpsimd"
    av_bufs = cfg.get("av_bufs", 1)
    scp = ctx.enter_context(
        tc.tile_pool(name="scp", bufs=cfg["scp_bufs"], space="PSUM"))
    avp = ctx.enter_context(
        tc.tile_pool(name="avp", bufs=av_bufs, space="PSUM"))
    utilp = ctx.enter_context(
        tc.tile_pool(name="utilp",
                     bufs=1 if (gp_rowsum or av_bufs > 1) else 2,
                     space="PSUM"))
    bcp = None
    if not gp_rowsum:
        bcp = ctx.enter_context(tc.tile_pool(name="bcp", bufs=1, space="PSUM"))

    # ---- constants ----
    idt = consts.tile([128, 128], f32, tag="idt")
    make_identity(nc, idt[:])
    ones = consts.tile([128, 128], mdt, tag="ones")
    nc.gpsimd.memset(ones[:], 1.0)

    # ---- load X and transpose into XT [d, n] (stored in matmul dtype) ----
    X_stage = big.tile([128, N], f32, tag="xstage")
    for nt in range(NT):
        nc.sync.dma_start(out=X_stage[:, nt * 128:(nt + 1) * 128],
                          in_=X[nt * 128:(nt + 1) * 128, :])
    XT = big.tile([128, N], mdt, tag="xt")
    for nt in range(NT):
        pt = utilp.tile([128, 128], f32, tag="u")
        nc.tensor.transpose(pt[:], X_stage[:, nt * 128:(nt + 1) * 128], idt[:])
        nc.vector.tensor_copy(XT[:, nt * 128:(nt + 1) * 128], pt[:])

    # ---- load W, V and cast ----
    Wf = big.tile([128, HC * 128], f32, tag="wf")
    Vf = big.tile([128, HC * 128], f32, tag="vf")
    for h in range(HC):
        nc.sync.dma_start(out=Wf[:, h * 128:(h + 1) * 128], in_=W[h])
        nc.sync.dma_start(out=Vf[:, h * 128:(h + 1) * 128], in_=V[h])
    Wc = big.tile([128, HC * 128], mdt, tag="wc")
    Vc = big.tile([128, HC * 128], mdt, tag="vc")
    nc.vector.tensor_copy(Wc[:], Wf[:])
    nc.vector.tensor_copy(Vc[:], Vf[:])

    # ---- XV for all heads: XV[m, e], tiled [mt][128, HC*128] ----
    XV = big.tile([128, NT * HC * 128], mdt, tag="xv")
    for mt in range(NT):
        sct = scp.tile([128, 1024], f32, tag="sc")
        nc.tensor.matmul(sct[:, 0:512], XT[:, mt * 128:(mt + 1) * 128],
                         Vc[:, 0:512], start=True, stop=True)
        nc.tensor.matmul(sct[:, 512:512 + (HC - 4) * 128],
                         XT[:, mt * 128:(mt + 1) * 128],
                         Vc[:, 512:HC * 128], start=True, stop=True)
        nc.vector.tensor_copy(XV[:, mt * HC * 128:(mt + 1) * HC * 128],
                              sct[:, 0:HC * 128])

    OUT_acc = big.tile([128, N], f32, tag="oacc")

    for h in range(HC):
        # ---- XWT[e, n] for this head ----
        XWT = xwtp.tile([128, N], mdt, tag="xwt")
        for g in range(2):
            sct = scp.tile([128, 1024], f32, tag="sc")
            for j in range(2):
                c = 2 * g + j
                nc.tensor.matmul(sct[:, j * 512:(j + 1) * 512],
                                 Wc[:, h * 128:(h + 1) * 128],
                                 XT[:, c * 512:(c + 1) * 512],
                                 start=True, stop=True)
            nc.vector.tensor_copy(XWT[:, g * 1024:(g + 1) * 1024], sct[:, 0:1024])

        for c in range(CH):
            ncol = slice(c * 512, (c + 1) * 512)
            EXP = expp.tile([128, NT * 512], mdt, tag="exp")
            if cfg["sched"] == "pipelined" and cfg["rowsum"] != "pe":
                # software-pipelined: AV matmuls and DVE exp-accumulation for
                # pair p-1 are emitted between the scores/exp of pair p, so
                # the PE never sits in-order behind a not-yet-finished exp.
                EACC = smallp.tile([128, 512], f32, tag="eacc")
                AV = avp.tile([128, 512], f32, tag="av")
                npairs = NT // 2
                for p in range(npairs + 1):
                    if p < npairs:
                        sct = scp.tile([128, 1024], f32, tag="sc")
                        for j in range(2):
                            mt = 2 * p + j
                            nc.tensor.matmul(sct[:, j * 512:(j + 1) * 512],
                                             XT[:, mt * 128:(mt + 1) * 128],
                                             XWT[:, ncol],
                                             start=True, stop=True)
                        nc.scalar.activation(EXP[:, p * 1024:(p + 1) * 1024],
                                             sct[:, 0:1024], Exp, scale=SCALE)
                    if p >= 1:
                        for j in range(2):
                            mt = 2 * (p - 1) + j
                            nc.tensor.matmul(
                                AV[:],
                                XV[:, mt * HC * 128 + h * 128:
                                       mt * HC * 128 + (h + 1) * 128],
                                EXP[:, mt * 512:(mt + 1) * 512],
                                start=(mt == 0), stop=(mt == NT - 1))
                            if mt == 0:
                                nc.vector.tensor_copy(EACC[:], EXP[:, 0:512])
                            else:
                                nc.vector.tensor_add(
                                    EACC[:], EACC[:],
                                    EXP[:, mt * 512:(mt + 1) * 512])
                if gp_rowsum:
                    from concourse import bass_isa
                    BCf = smallp.tile([128, 512], f32, tag="bcf")
                    nc.gpsimd.partition_all_reduce(BCf[:], EACC[:], 128,
                                                   bass_isa.ReduceOp.add)
                    BC_sb = smallp.tile([128, 512], f32, tag="bcsb")
                    nc.vector.reciprocal(BC_sb[:], BCf[:])
                else:
                    EACCh = smallp.tile([128, 512], mdt, tag="eacch")
                    nc.vector.tensor_copy(EACCh[:], EACC[:])
                    RS = utilp.tile([1, 512], f32, tag="u")
                    nc.tensor.matmul(RS[:], ones[:, 0:1], EACCh[:],
                                     start=True, stop=True)
                    RSr = smallp.tile([1, 512], f32, tag="rsr")
                    nc.vector.reciprocal(RSr[:], RS[:])
                    RSh = smallp.tile([1, 512], mdt, tag="rsh")
                    nc.vector.tensor_copy(RSh[:], RSr[:])
                    BC = bcp.tile([128, 512], f32, tag="bc")
                    nc.tensor.matmul(BC[:], ones[0:1, :], RSh[:],
                                     start=True, stop=True)
                    BC_sb = smallp.tile([128, 512], f32, tag="bcsb")
                    nc.vector.tensor_copy(BC_sb[:], BC[:])
                if h == 0:
                    nc.vector.tensor_mul(OUT_acc[:, ncol], AV[:], BC_sb[:])
                else:
                    tmp = smallp.tile([128, 512], f32, tag="tmp")
                    nc.vector.tensor_mul(tmp[:], AV[:], BC_sb[:])
                    nc.vector.tensor_add(OUT_acc[:, ncol], OUT_acc[:, ncol],
                                         tmp[:])
                continue
            # scores (transposed) + exp
            if cfg["scpsum"] == "f16":
                # 4 m-tiles of f16 scores per 2-bank PSUM tile, one exp each
                for q in range(NT // 4):
                    sct = scp.tile([128, 2048], mdt, tag="sc")
                    for j in range(4):
                        mt = 4 * q + j
                        nc.tensor.matmul(sct[:, j * 512:(j + 1) * 512],
                                         XT[:, mt * 128:(mt + 1) * 128],
                                         XWT[:, ncol],
                                         start=True, stop=True)
                    nc.scalar.activation(EXP[:, q * 2048:(q + 1) * 2048],
                                         sct[:, 0:2048], Exp, scale=SCALE)
            else:
                for p in range(NT // 2):
                    sct = scp.tile([128, 1024], f32, tag="sc")
                    for j in range(2):
                        mt = 2 * p + j
                        nc.tensor.matmul(sct[:, j * 512:(j + 1) * 512],
                                         XT[:, mt * 128:(mt + 1) * 128],
                                         XWT[:, ncol],
                                         start=True, stop=True)
                    nc.scalar.activation(EXP[:, p * 1024:(p + 1) * 1024],
                                         sct[:, 0:1024], Exp, scale=SCALE)
            # softmax denominator -> per-partition broadcast reciprocal BC_sb
            if cfg["rowsum"] == "pe":
                RS = utilp.tile([1, 512], f32, tag="u")
                for mt in range(NT):
                    nc.tensor.matmul(RS[:], ones[:, 0:1],
                                     EXP[:, mt * 512:(mt + 1) * 512],
                                     start=(mt == 0), stop=(mt == NT - 1))
                RSr = smallp.tile([1, 512], f32, tag="rsr")
                nc.vector.reciprocal(RSr[:], RS[:])
                RSh = smallp.tile([1, 512], mdt, tag="rsh")
                nc.vector.tensor_copy(RSh[:], RSr[:])
                BC = bcp.tile([128, 512], f32, tag="bc")
                nc.tensor.matmul(BC[:], ones[0:1, :], RSh[:],
                                 start=True, stop=True)
                BC_sb = smallp.tile([128, 512], f32, tag="bcsb")
                nc.vector.tensor_copy(BC_sb[:], BC[:])
            else:
                EACC = smallp.tile([128, 512], f32, tag="eacc")
                if cfg["rowsum"] == "dve_reduce":
                    # view EXP as [p][n=512][mt=16] and reduce innermost
                    ev = EXP.rearrange("p (mt n) -> p n mt", mt=NT)
                    nc.vector.tensor_reduce(EACC[:], ev,
                                            axis=mybir.AxisListType.X,
                                            op=mybir.AluOpType.add)
                else:
                    nc.vector.tensor_copy(EACC[:], EXP[:, 0:512])
                    for mt in range(1, NT):
                        nc.vector.tensor_add(EACC[:], EACC[:],
                                             EXP[:, mt * 512:(mt + 1) * 512])
                if gp_rowsum:
                    # all partitions receive the partition-sum -> reciprocal
                    # IS the broadcast; no PE/PSUM round-trip needed
                    from concourse import bass_isa
                    BCf = smallp.tile([128, 512], f32, tag="bcf")
                    nc.gpsimd.partition_all_reduce(BCf[:], EACC[:], 128,
                                                   bass_isa.ReduceOp.add)
                    BC_sb = smallp.tile([128, 512], f32, tag="bcsb")
                    nc.vector.reciprocal(BC_sb[:], BCf[:])
                else:
                    EACCh = smallp.tile([128, 512], mdt, tag="eacch")
                    nc.vector.tensor_copy(EACCh[:], EACC[:])
                    RS = utilp.tile([1, 512], f32, tag="u")
                    nc.tensor.matmul(RS[:], ones[:, 0:1], EACCh[:],
                                     start=True, stop=True)
                    RSr = smallp.tile([1, 512], f32, tag="rsr")
                    nc.vector.reciprocal(RSr[:], RS[:])
                    RSh = smallp.tile([1, 512], mdt, tag="rsh")
                    nc.vector.tensor_copy(RSh[:], RSr[:])
                    BC = bcp.tile([128, 512], f32, tag="bc")
                    nc.tensor.matmul(BC[:], ones[0:1, :], RSh[:],
                                     start=True, stop=True)
                    BC_sb = smallp.tile([128, 512], f32, tag="bcsb")
                    nc.vector.tensor_copy(BC_sb[:], BC[:])
            # AV (transposed): avT[e, n] accumulated over m tiles
            AV = avp.tile([128, 512], f32, tag="av")
            for mt in range(NT):
                nc.tensor.matmul(AV[:],
                                 XV[:, mt * HC * 128 + h * 128:
                                        mt * HC * 128 + (h + 1) * 128],
                                 EXP[:, mt * 512:(mt + 1) * 512],
                                 start=(mt == 0), stop=(mt == NT - 1))
            # normalise + accumulate over heads
            if h == 0:
                nc.vector.tensor_mul(OUT_acc[:, ncol], AV[:], BC_sb[:])
            else:
                tmp = smallp.tile([128, 512], f32, tag="tmp")
                nc.vector.tensor_mul(tmp[:], AV[:], BC_sb[:])
                nc.vector.tensor_add(OUT_acc[:, ncol], OUT_acc[:, ncol], tmp[:])

    # ---- transpose OUT_acc [e, n] -> out [n, e] and store ----
    for nt in range(NT):
        pt = utilp.tile([128, 128], f32, tag="u")
        nc.tensor.transpose(pt[:], OUT_acc[:, nt * 128:(nt + 1) * 128], idt[:])
        OUTN = smallp.tile([128, 128], f32, tag="outn")
        nc.vector.tensor_copy(OUTN[:], pt[:])
        nc.sync.dma_start(out=out[nt * 128:(nt + 1) * 128, :], in_=OUTN[:])


def build(num_devices=NCORES, cfg=None, reps=None):
    import concourse.bacc as bacc
    import concourse.tile as tile
    from concourse import mybir
    from contextlib import ExitStack

    cfg = dict(CFG, **(cfg or {}))
    nc = bacc.Bacc("TRN2", target_bir_lowering=False, debug=False,
                   num_devices=num_devices)
    f32 = mybir.dt.float32
    X = nc.dram_tensor("X", [N, D], f32, kind="ExternalInput").ap()
    W = nc.dram_tensor("W", [HC, D, D], f32, kind="ExternalInput").ap()
    V = nc.dram_tensor("V", [HC, D, D], f32, kind="ExternalInput").ap()
    out = nc.dram_tensor("out", [N, D], f32, kind="ExternalOutput").ap()
    with tile.TileContext(nc) as tc:
        with ExitStack() as ctx:
            if reps:
                # benchmark mode: run the body `reps` times on-device
                with tc.For_i(0, reps, 1):
                    _emit(ctx, tc, nc, X, W, V, out, cfg)
            else:
                _emit(ctx, tc, nc, X, W, V, out, cfg)
    nc.compile()
    return nc


def _get_nc():
    key = tuple(sorted(CFG.items()))
    if key not in _CACHE:
        _CACHE[key] = build()
    return _CACHE[key]


def kernel(X, W, V):
    from concourse.bass_utils import run_bass_kernel_spmd

    X = np.ascontiguousarray(np.asarray(X, dtype=np.float32))
    W = np.ascontiguousarray(np.asarray(W, dtype=np.float32))
    V = np.ascontiguousarray(np.asarray(V, dtype=np.float32))
    nc = _get_nc()
    in_maps = [
        {"X": X,
         "W": np.ascontiguousarray(W[c * HC:(c + 1) * HC]),
         "V": np.ascontiguousarray(V[c * HC:(c + 1) * HC])}
        for c in range(NCORES)
    ]
    res = run_bass_kernel_spmd(nc, in_maps, list(range(NCORES)))
    partials = np.stack([res.results[c]["out"] for c in range(NCORES)])
    return partials.sum(axis=0, dtype=np.float32)

